# revision 1
# baseline (speedup 1.0000x reference)
"""Trainium2 kernel for nn_Net_19086834664186.

The reference net is Linear(55, 55) followed by a 300-step Euler
integration of a DMP (dynamic movement primitive). The DMP phase
variable and basis activations are batch-independent and the Euler
recurrence is linear in (y0, goal, forcing weights), so the entire
integration folds into a constant coefficient matrix C (27, 301)
computed once on the host in float64. Composing with the Linear layer
gives out_flat = [x | 1] @ Gp with Gp (56, 602); the device runs only
that matmul, sharded over the batch across 8 cores (pure data
parallel), which is store-bandwidth bound exactly like the reference.

Device layout per core (shard = 8192 rows):
  - xT (56, 8192): transposed shard of [x | 1]; columns permuted so that
    each store-group's output rows land contiguously per SBUF partition.
  - G (56, 602): folded weights (replicated).
  - 64 chunks of 128 rows: matmul (56,128)^T @ (56,602) -> psum (128,602),
    DVE copy psum->sbuf, one DMA store per STORE_GROUP chunks whose
    per-partition destination is a single contiguous DRAM run.
"""
import numpy as np

import concourse.bass as bass
import concourse.bacc as bacc
import concourse.mybir as mybir
from concourse.tile import TileContext
from concourse.bass_utils import run_bass_kernel_spmd

# --- DMP constants (from Net.__init__ / DMP_integrator(25, 3, 0.01, 2, 1.0)) ---
N_BASIS = 25
TAU = 3.0
DT = 0.01
DOF = 2
A_Z = 48.0
B_Z = A_Z / 4.0
A_X = 2.0
T_STEPS = 300
SCALE = 1.0
K_EUL = DT / TAU

BATCH = 65536
N_CORES = 8
SHARD = BATCH // N_CORES          # 8192 rows per core
KDIM = 56                         # 55 features + 1 bias column
NOUT = 2 * (T_STEPS + 1)          # 602
P = 128                           # rows per matmul chunk
CHUNKS = SHARD // P               # 64

# tunables
STORE_GROUP = 8                   # chunks per store DMA
CONTIG_STORE = True               # permute rows so stores are contiguous/partition
OPOOL_BUFS = 6
PPOOL_BUFS = 4
XLOAD_SPLIT = 4
MM_DTYPE = "f32r"                 # tf32-like PE path: 2x fp32 throughput,
                                  # absmax err ~7e-4 on out scale 3.7

_FP32 = mybir.dt.float32


def _coeff_matrix(dtype=np.float64):
    """C: (27, 301). Row basis [y0, g, w_0..w_24] -> y_t for t = 0..300."""
    c = np.exp(-A_X * np.linspace(0.0, 1.0, N_BASIS, dtype=dtype))
    s = np.diff(c) * dtype(0.75)
    sigma2 = np.concatenate([s, s[-1:]]) ** 2

    C = np.zeros((2 + N_BASIS, T_STEPS + 1), dtype=dtype)
    Y = np.zeros(2 + N_BASIS, dtype=dtype)
    Z = np.zeros(2 + N_BASIS, dtype=dtype)
    Y[0] = 1.0
    C[:, 0] = Y
    e_g = np.zeros(2 + N_BASIS, dtype=dtype)
    e_g[1] = 1.0

    xp = dtype(1.0)
    for t in range(T_STEPS):
        psi = np.exp(-0.5 * (xp - c) ** 2 / sigma2)
        fx = np.zeros(2 + N_BASIS, dtype=dtype)
        fx[2:] = SCALE * psi * (xp / psi.sum())
        dz = (A_Z * (B_Z * (e_g - Y) - Z) + fx) * K_EUL
        Y = Y + Z * K_EUL
        Z = Z + dz
        xp = xp - A_X * xp * K_EUL
        C[:, t + 1] = Y
    return C


def _fold_weights(W, b):
    """Gp (56, 602) with out_flat = [x | 1] @ Gp; h slots [tau, y0(2), g(2), w(50)]."""
    C = _coeff_matrix()
    W64 = np.asarray(W).astype(np.float64)
    b64 = np.asarray(b).astype(np.float64)
    Gp = np.zeros((KDIM, NOUT), dtype=np.float64)
    for d in range(DOF):
        idx = [1 + d, 3 + d] + list(range(5 + N_BASIS * d, 5 + N_BASIS * (d + 1)))
        Gp[:55, d * 301:(d + 1) * 301] = W64[idx, :].T @ C
        Gp[55, d * 301:(d + 1) * 301] = b64[idx] @ C
    return np.ascontiguousarray(Gp.astype(np.float32))


def _prep_in_maps(x, W, b, contig=CONTIG_STORE, store_group=STORE_GROUP,
                  mm_dtype=MM_DTYPE):
    """Host-side prep: fold weights, transpose+augment x, shard (and permute
    columns so each store group's rows are partition-contiguous)."""
    x = np.ascontiguousarray(x, dtype=np.float32)
    Gp = _fold_weights(W, b)
    np_dt = np.float32
    if mm_dtype == "bf16":
        import ml_dtypes
        np_dt = ml_dtypes.bfloat16
        Gp = Gp.astype(np_dt)
    xa = np.empty((KDIM, BATCH), dtype=np_dt)
    xa[:55] = x.T
    xa[55] = 1.0
    in_maps = []
    n_g = CHUNKS // store_group
    for i in range(N_CORES):
        shard = xa[:, i * SHARD:(i + 1) * SHARD]
        if contig:
            # natural col = s*(128*g) + p*g + j  ->  permuted col = s*(128*g) + j*128 + p
            shard = np.ascontiguousarray(
                shard.reshape(KDIM, n_g, P, store_group)
                .transpose(0, 1, 3, 2)
                .reshape(KDIM, SHARD))
        else:
            shard = np.ascontiguousarray(shard)
        in_maps.append({"xT": shard, "G": Gp})
    return in_maps


def _build_nc(reps=1, loop_n=None, store_group=STORE_GROUP, contig=CONTIG_STORE,
              opool_bufs=OPOOL_BUFS, ppool_bufs=PPOOL_BUFS,
              xload_split=XLOAD_SPLIT, pair_copy=False, store_only=False,
              copy_mode="dve", mm_dtype=MM_DTYPE, store_eng="sync"):
    n_groups = CHUNKS // store_group
    _in_dt = {"bf16": mybir.dt.bfloat16,
              "f32r": mybir.dt.float32r,
              "f32": _FP32}[mm_dtype]
    _mm_cast = lambda ap: ap
    nc = bacc.Bacc(None, target_bir_lowering=False)
    xT = nc.dram_tensor("xT", [KDIM, SHARD], _in_dt, kind="ExternalInput")
    G = nc.dram_tensor("G", [KDIM, NOUT], _in_dt, kind="ExternalInput")
    out = nc.dram_tensor("out", [SHARD, NOUT], _FP32, kind="ExternalOutput")

    if contig:
        # partition p of group s holds rows s*(128*g)+p*g+j, j=0..g-1:
        # per-partition destination is one contiguous run of g*602 floats
        out_v = out.rearrange("(s p j) t -> s p (j t)", p=P, j=store_group)
    else:
        # row = (s*g + c)*128 + p
        out_v = out.rearrange("(s c p) t -> s p c t", c=store_group, p=P)

    with TileContext(nc) as tc:
        with (
            tc.tile_pool(name="const", bufs=1) as cpool,
            tc.tile_pool(name="outp", bufs=opool_bufs) as opool,
            tc.tile_pool(name="ps", bufs=ppool_bufs, space="PSUM") as ppool,
        ):
            g = cpool.tile([KDIM, NOUT], _in_dt)
            nc.sync.dma_start(g[:], G[:])
            x = cpool.tile([KDIM, SHARD], _in_dt)
            for i in range(xload_split):
                nc.sync.dma_start(x[:, bass.ts(i, SHARD // xload_split)],
                                  xT[:, bass.ts(i, SHARD // xload_split)])

            def body():
                for s in range(n_groups):
                    if store_eng == "gp":
                        _store = nc.gpsimd.dma_start
                    else:
                        _store = nc.sync.dma_start if (store_eng == "sync"
                                                       or s % 2 == 0) \
                            else nc.scalar.dma_start
                    o = opool.tile([P, store_group, NOUT], _FP32, name="o")
                    if store_only:
                        # ablation: measure pure store bandwidth
                        nc.vector.memset(o[:, 0, 0:8], 0.0)
                        _store(out_v[s], o[:])
                        continue
                    if pair_copy:
                        # two chunks per 4-bank psum tile; one DVE copy per pair
                        for cp in range(store_group // 2):
                            ps = ppool.tile([P, 2048], _FP32, name="ps",
                                            bufs=2)
                            for h in range(2):
                                chunk = s * store_group + cp * 2 + h
                                lhsT = x[:, bass.ts(chunk, P)]
                                base = h * 1024
                                nc.tensor.matmul(ps[:, base:base + 512],
                                                 _mm_cast(lhsT),
                                                 _mm_cast(g[:, 0:512]),
                                                 start=True, stop=True)
                                nc.tensor.matmul(ps[:, base + 512:base + NOUT],
                                                 _mm_cast(lhsT),
                                                 _mm_cast(g[:, 512:NOUT]),
                                                 start=True, stop=True)
                            src = ps[:, :].rearrange("p (h q) -> p h q", h=2)
                            nc.vector.tensor_copy(
                                o[:, cp * 2:cp * 2 + 2, :], src[:, :, 0:NOUT])
                    else:
                        for c in range(store_group):
                            chunk = s * store_group + c
                            ps = ppool.tile([P, NOUT], _FP32, name="ps")
                            lhsT = x[:, bass.ts(chunk, P)]  # (56,128) stationary
                            nc.tensor.matmul(ps[:, 0:512], _mm_cast(lhsT),
                                             _mm_cast(g[:, 0:512]),
                                             start=True, stop=True)
                            nc.tensor.matmul(ps[:, 512:NOUT], _mm_cast(lhsT),
                                             _mm_cast(g[:, 512:NOUT]),
                                             start=True, stop=True)
                            if copy_mode == "dve":
                                nc.vector.tensor_copy(o[:, c, :], ps[:])
                            elif copy_mode == "act":
                                nc.scalar.copy(o[:, c, :], ps[:])
                            elif copy_mode == "alt":
                                eng = nc.vector.tensor_copy if c % 2 == 0 \
                                    else nc.scalar.copy
                                eng(o[:, c, :], ps[:])
                            elif copy_mode == "alt3":
                                eng = nc.scalar.copy if c % 3 == 2 \
                                    else nc.vector.tensor_copy
                                eng(o[:, c, :], ps[:])
                            elif copy_mode == "dve2":
                                nc.vector.tensor_copy(o[:, c, 0:512],
                                                      ps[:, 0:512])
                                nc.vector.tensor_copy(o[:, c, 512:NOUT],
                                                      ps[:, 512:NOUT])
                            else:
                                raise ValueError(copy_mode)
                    _store(out_v[s], o[:])

            if loop_n is not None:
                with tc.For_i(0, loop_n, 1):
                    body()
            else:
                for _rep in range(reps):
                    body()
    nc.compile()
    return nc


_CACHED_NC = None


def _get_nc():
    global _CACHED_NC
    if _CACHED_NC is None:
        _CACHED_NC = _build_nc()
    return _CACHED_NC


def kernel(x, W, b, _spmd_kwargs=None):
    in_maps = _prep_in_maps(x, W, b)
    res = run_bass_kernel_spmd(_get_nc(), in_maps, list(range(N_CORES)),
                               **(_spmd_kwargs or {}))
    out = np.concatenate([r["out"] for r in res.results], axis=0)
    if _spmd_kwargs:
        kernel.last_results = res
    return out.reshape(BATCH, DOF, T_STEPS + 1)

